# revision 20
# baseline (speedup 1.0000x reference)
"""Trainium2 Bass kernel for nn_Network_85091892069069 (dense_mlp).

MLP: x[N,32] -> Linear(32,64)+ReLU -> Linear(64,64)+ReLU -> Linear(64,64) -> y[N,64]
N = 1048576, f32. Pure data parallel over 8 NeuronCores (131072 rows/core).

Per-core scheme (feature-major activations):
  - DMA x supertile [2048,32] -> SBUF [128, 512] (16 row-blocks side by side)
  - PE transpose (4x [128,128]) -> PSUM xt [128,512]: partition 32q+d holds
    feature d of row-block (q,c), free = 128c+n
  - Block-diagonal weights make every layer a K=64/128, M=128, N=512 matmul
    (two matmuls per layer cover all 4 q-blocks), fp32r fast mode
  - Epilogue = bias+ReLU fused into the mandatory PSUM->SBUF move (ACT/DVE)
  - PE transpose back to row-major, DMA out [2048,64] contiguous
"""

import os
import sys
from contextlib import ExitStack

import numpy as np

for _p in ("/opt/trn_rl_repo",):
    if os.path.isdir(_p) and _p not in sys.path:
        sys.path.insert(0, _p)

import concourse.bacc as bacc
import concourse.bass as bass
import concourse.tile as tile
from concourse import mybir
from concourse import bass_utils

F32 = mybir.dt.float32
F32R = mybir.dt.float32r
AF = mybir.ActivationFunctionType
ALU = mybir.AluOpType

N_CORES = 8
N_TOTAL = 1048576
D_IN = 32
W = 64
ROWS_PER_CORE = N_TOTAL // N_CORES  # 131072
ST_ROWS = 2048  # rows per supertile

# knobs
MM_FAST = os.environ.get("KERNEL_MM_FAST", "1") == "1"  # fp32r matmuls
OUT_ROWMAJOR = os.environ.get("KERNEL_OUT_ROWMAJOR", "1") == "1"


def _emit(ctx: ExitStack, nc: bass.Bass, tc: tile.TileContext, aps, rows_per_core):
    x, y, w1, w2, w3, ident, b1, b2, b3 = aps
    nst = rows_per_core // ST_ROWS

    MMDT = F32R if MM_FAST else F32

    consts = ctx.enter_context(tc.tile_pool(name="consts", bufs=1))
    w1_sb = consts.tile([128, 128], MMDT, tag="w1")
    w2_sb = consts.tile([128, 128], MMDT, tag="w2")
    w3_sb = consts.tile([128, 128], MMDT, tag="w3")
    id_sb = consts.tile([128, 128], F32, tag="ident")
    b1_sb = consts.tile([128, 1], F32, tag="b1")
    b2_sb = consts.tile([128, 1], F32, tag="b2")
    b3_sb = consts.tile([128, 1], F32, tag="b3")
    wdma = nc.gpsimd.dma_start if MM_FAST else nc.sync.dma_start
    wdma(w1_sb[:], w1)
    wdma(w2_sb[:], w2)
    wdma(w3_sb[:], w3)
    nc.sync.dma_start(id_sb[:], ident)
    nc.sync.dma_start(b1_sb[:], b1)
    nc.sync.dma_start(b2_sb[:], b2)
    nc.sync.dma_start(b3_sb[:], b3)

    pool_in = ctx.enter_context(tc.tile_pool(name="xin", bufs=3))
    pool_xt = ctx.enter_context(tc.tile_pool(name="xt", bufs=3))
    pool_h = ctx.enter_context(tc.tile_pool(name="h", bufs=4))
    pool_ps_xt = ctx.enter_context(tc.tile_pool(name="ps_xt", bufs=2, space="PSUM"))
    pool_ps = ctx.enter_context(tc.tile_pool(name="ps", bufs=2, space="PSUM"))
    if OUT_ROWMAJOR:
        pool_out = ctx.enter_context(tc.tile_pool(name="oT", bufs=3))

    # Partition p of the x tile holds rows 16p..16p+15 (2KB contiguous DRAM).
    # After the PE transpose of chunk c, ps_xt[32q+d, 128c+n] = feature d of
    # row base + 16n + 4c + q, i.e. block b=4c+q covers rows {16n + b}.
    x_r = x.rearrange("(t p w) d -> t p (w d)", p=128, w=16)
    if OUT_ROWMAJOR:
        # oT[n, 64b+m] -> y[base + 16n + b, m]; per-partition 4KB contiguous
        y_r = y.rearrange("(t n b) m -> t n (b m)", n=128, b=16)
    else:
        # raw device layout [128, rows/2]; host unscrambles (see _run)
        y_r = y.rearrange("p (t f) -> t p f", f=1024)

    for t in range(nst):
        xa = pool_in.tile([128, 512], F32, tag="xa")
        nc.sync.dma_start(xa[:], x_r[t])

        # transpose to feature-major: ps_xt[32q+d, 128c+n] = x[base+128*(4c+q)+n, d]
        ps_xt = pool_ps_xt.tile([128, 512], F32, tag="psxt")
        for c in range(4):
            nc.tensor.transpose(
                ps_xt[:, 128 * c : 128 * (c + 1)],
                xa[:, 128 * c : 128 * (c + 1)],
                id_sb[:],
            )
        xt = pool_xt.tile([128, 512], MMDT, tag="xt")
        nc.vector.tensor_copy(xt[:], ps_xt[:])

        # layer 1: q-blocks {0,1} from xt[0:64], {2,3} from xt[64:128]
        ps1 = pool_ps.tile([128, 1024], F32, tag="ps")
        nc.tensor.matmul(ps1[:, 0:512], w1_sb[0:64, :], xt[0:64, :], start=True, stop=True)
        nc.tensor.matmul(ps1[:, 512:1024], w1_sb[64:128, :], xt[64:128, :], start=True, stop=True)
        h1 = pool_h.tile([128, 1024], MMDT, tag="h")
        nc.scalar.activation(h1[:], ps1[:], AF.Relu, bias=b1_sb[:])

        # layer 2
        ps2 = pool_ps.tile([128, 1024], F32, tag="ps")
        nc.tensor.matmul(ps2[:, 0:512], w2_sb[:], h1[:, 0:512], start=True, stop=True)
        nc.tensor.matmul(ps2[:, 512:1024], w2_sb[:], h1[:, 512:1024], start=True, stop=True)
        h2 = pool_h.tile([128, 1024], MMDT, tag="h")
        nc.scalar.activation(h2[:], ps2[:], AF.Relu, bias=b2_sb[:])

        # layer 3 (no relu): bias-add fused into PSUM->SBUF move on DVE
        ps3 = pool_ps.tile([128, 1024], F32, tag="ps")
        nc.tensor.matmul(ps3[:, 0:512], w3_sb[:], h2[:, 0:512], start=True, stop=True)
        nc.tensor.matmul(ps3[:, 512:1024], w3_sb[:], h2[:, 512:1024], start=True, stop=True)
        o = pool_h.tile([128, 1024], F32, tag="h")
        nc.vector.tensor_scalar_add(o[:], ps3[:], b3_sb[:])

        if OUT_ROWMAJOR:
            # transpose back: chunk (X, c) of o -> ps_oT chunk 4X+c
            ps_oT = pool_ps.tile([128, 1024], F32, tag="ps")
            for X in range(2):
                for c in range(4):
                    k = 4 * X + c
                    nc.tensor.transpose(
                        ps_oT[:, 128 * k : 128 * (k + 1)],
                        o[:, 512 * X + 128 * c : 512 * X + 128 * (c + 1)],
                        id_sb[:],
                    )
            # scatter-copy to oT[n, 64b+m], b = 4c + 2X + j:
            #   chunk (X, c) holds [n, (j m)] -> oT cols 256c + 128X + [0,128)
            oT = pool_out.tile([128, 1024], F32, tag="oT")
            oT4 = oT[:].rearrange("n (c x jm) -> n c x jm", c=4, x=2)
            psoT3 = ps_oT[:].rearrange("n (x c jm) -> n x c jm", x=2, c=4)
            # ACT does X=0 half, DVE does X=1 half (balance engines)
            nc.scalar.copy(oT4[:, :, 0, :], psoT3[:, 0])
            nc.vector.tensor_copy(oT4[:, :, 1, :], psoT3[:, 1])
            nc.sync.dma_start(y_r[t], oT[:])
        else:
            nc.sync.dma_start(y_r[t], o[:])


def _build_program(rows_per_core=ROWS_PER_CORE, repeats=1):
    nc = bacc.Bacc(
        "TRN2",
        target_bir_lowering=False,
        debug=False,
        num_devices=N_CORES,
    )
    x = nc.dram_tensor("x", [rows_per_core, D_IN], F32, kind="ExternalInput").ap()
    if OUT_ROWMAJOR:
        y = nc.dram_tensor("y", [rows_per_core, W], F32, kind="ExternalOutput").ap()
    else:
        y = nc.dram_tensor("y", [128, rows_per_core // 2], F32, kind="ExternalOutput").ap()
    w1 = nc.dram_tensor("w1blk", [128, 128], F32, kind="ExternalInput").ap()
    w2 = nc.dram_tensor("w2blk", [128, 128], F32, kind="ExternalInput").ap()
    w3 = nc.dram_tensor("w3blk", [128, 128], F32, kind="ExternalInput").ap()
    ident = nc.dram_tensor("ident", [128, 128], F32, kind="ExternalInput").ap()
    b1 = nc.dram_tensor("b1bc", [128, 1], F32, kind="ExternalInput").ap()
    b2 = nc.dram_tensor("b2bc", [128, 1], F32, kind="ExternalInput").ap()
    b3 = nc.dram_tensor("b3bc", [128, 1], F32, kind="ExternalInput").ap()
    aps = (x, y, w1, w2, w3, ident, b1, b2, b3)
    with tile.TileContext(nc) as tc:
        with ExitStack() as ctx:
            if repeats > 1:
                with tc.For_i(0, repeats, 1):
                    _emit(ctx, nc, tc, aps, rows_per_core)
            else:
                _emit(ctx, nc, tc, aps, rows_per_core)
    nc.compile()
    return nc


def _host_blocks(W1, b1, W2, b2, W3, b3):
    w1blk = np.zeros((128, 128), np.float32)
    w1blk[0:32, 0:64] = W1
    w1blk[32:64, 64:128] = W1
    w1blk[64:96, 0:64] = W1
    w1blk[96:128, 64:128] = W1
    w2blk = np.zeros((128, 128), np.float32)
    w2blk[0:64, 0:64] = W2
    w2blk[64:128, 64:128] = W2
    w3blk = np.zeros((128, 128), np.float32)
    w3blk[0:64, 0:64] = W3
    w3blk[64:128, 64:128] = W3
    ident = np.eye(128, dtype=np.float32)
    b1bc = np.ascontiguousarray(np.concatenate([b1, b1])[:, None].astype(np.float32))
    b2bc = np.ascontiguousarray(np.concatenate([b2, b2])[:, None].astype(np.float32))
    b3bc = np.ascontiguousarray(np.concatenate([b3, b3])[:, None].astype(np.float32))
    return dict(
        w1blk=w1blk, w2blk=w2blk, w3blk=w3blk, ident=ident,
        b1bc=b1bc, b2bc=b2bc, b3bc=b3bc,
    )


_NC_CACHE = {}


def _get_program(rows_per_core=ROWS_PER_CORE, repeats=1):
    key = (rows_per_core, MM_FAST, OUT_ROWMAJOR, repeats)
    if key not in _NC_CACHE:
        _NC_CACHE[key] = _build_program(rows_per_core, repeats=repeats)
    return _NC_CACHE[key]


def _run(inputs, rows_per_core=ROWS_PER_CORE, trace=False, trace_kwargs=None):
    nc = _get_program(rows_per_core)
    blocks = _host_blocks(
        inputs["W1"], inputs["b1"], inputs["W2"], inputs["b2"],
        inputs["W3"], inputs["b3"],
    )
    x = np.ascontiguousarray(np.asarray(inputs["x"], dtype=np.float32))
    n_rows = x.shape[0]
    assert n_rows == rows_per_core * N_CORES
    in_maps = []
    for c in range(N_CORES):
        m = dict(blocks)
        m["x"] = np.ascontiguousarray(x[c * rows_per_core : (c + 1) * rows_per_core])
        in_maps.append(m)
    kw = {}
    if trace:
        kw["trace"] = True
        if trace_kwargs:
            kw.update(trace_kwargs)
    res = bass_utils.run_bass_kernel_spmd(nc, in_maps, core_ids=list(range(N_CORES)), **kw)
    outs = [r["y"] for r in res.results]
    if OUT_ROWMAJOR:
        full = np.concatenate(outs, axis=0)
    else:
        nst = rows_per_core // ST_ROWS
        decoded = []
        for o in outs:
            # o[64j+m, 1024t + 512X + 128c + n] = y[2048t + 16n + 4c + 2X + j, m]
            a = o.reshape(2, 64, nst, 2, 4, 128)  # j, m, t, X, c, n
            a = a.transpose(2, 5, 4, 3, 0, 1)  # t, n, c, X, j, m
            decoded.append(np.ascontiguousarray(a.reshape(rows_per_core, 64)))
        full = np.concatenate(decoded, axis=0)
    return np.ascontiguousarray(full.astype(np.float32)), res


def kernel(x, W1, b1, W2, b2, W3, b3):
    out, _ = _run(dict(x=x, W1=W1, b1=b1, W2=W2, b2=b2, W3=W3, b3=b3))
    return out


# revision 36
# speedup vs baseline: 2.8024x; 2.8024x over previous
"""Trainium2 Bass kernel for nn_Network_85091892069069 (dense_mlp).

MLP: x[N,32] -> Linear(32,64)+ReLU -> Linear(64,64)+ReLU -> Linear(64,64) -> y[N,64]
N = 1048576, f32. Pure data parallel over 8 NeuronCores (131072 rows/core).

Per-core scheme (feature-major activations):
  - DMA x supertile [2048,32] -> SBUF [128, 512] (16 row-blocks side by side)
  - PE transpose (4x [128,128]) -> PSUM xt [128,512]: partition 32q+d holds
    feature d of row-block (q,c), free = 128c+n
  - Block-diagonal weights make every layer a K=64/128, M=128, N=512 matmul
    (two matmuls per layer cover all 4 q-blocks), fp32r fast mode
  - Epilogue = bias+ReLU fused into the mandatory PSUM->SBUF move (ACT/DVE)
  - PE transpose back to row-major, DMA out [2048,64] contiguous
"""

import os
import sys
from contextlib import ExitStack

import numpy as np

for _p in ("/opt/trn_rl_repo",):
    if os.path.isdir(_p) and _p not in sys.path:
        sys.path.insert(0, _p)

import concourse.bacc as bacc
import concourse.bass as bass
import concourse.tile as tile
from concourse import mybir
from concourse import bass_utils

F32 = mybir.dt.float32
F32R = mybir.dt.float32r
AF = mybir.ActivationFunctionType
ALU = mybir.AluOpType

N_CORES = 8
N_TOTAL = 1048576
D_IN = 32
W = 64
ROWS_PER_CORE = N_TOTAL // N_CORES  # 131072
ST_ROWS = 2048  # rows per supertile

# knobs
MM_FAST = os.environ.get("KERNEL_MM_FAST", "1") == "1"  # fp32r matmuls
OUT_ROWMAJOR = os.environ.get("KERNEL_OUT_ROWMAJOR", "1") == "1"
# timing-only debug knobs (produce wrong results when set)
DBG_SKIP_INT = os.environ.get("DBG_SKIP_INT", "0") == "1"    # skip input transpose
DBG_SKIP_MM = os.environ.get("DBG_SKIP_MM", "0") == "1"      # skip layer matmuls+epilogues
DBG_SKIP_OUTT = os.environ.get("DBG_SKIP_OUTT", "0") == "1"  # skip output transpose path


def _emit(ctx: ExitStack, nc: bass.Bass, tc: tile.TileContext, aps, rows_per_core):
    """Software-pipelined emission: stage s of supertile t is emitted in wave
    w = t + SKEW[s], so each wave's PE/ACT/DVE work comes from different
    supertiles and every engine always has independent ready work."""
    x, y, w1, w2, w3, ident, b1, b2, b3 = aps
    nst = rows_per_core // ST_ROWS

    MMDT = F32R if MM_FAST else F32

    consts = ctx.enter_context(tc.tile_pool(name="consts", bufs=1))
    call = consts.tile([128, 515], MMDT, tag="call")
    nc.gpsimd.dma_start(call[:], w1)  # w1 arg = concatenated const block
    w1_sb = call[:, 0:128]
    w2_sb = call[:, 128:256]
    w3_sb = call[:, 256:384]
    id_sb = call[:, 384:512]
    id_f32 = id_sb.bitcast(F32)
    b1_sb = call[:, 512:513].bitcast(F32)
    b2_sb = call[:, 513:514].bitcast(F32)
    b3_sb = call[:, 514:515].bitcast(F32)

    pool_in = ctx.enter_context(tc.tile_pool(name="xin", bufs=6))
    pool_xt = ctx.enter_context(tc.tile_pool(name="xt", bufs=6))
    pool_h1 = ctx.enter_context(tc.tile_pool(name="h1", bufs=6))
    pool_h2 = ctx.enter_context(tc.tile_pool(name="h2", bufs=6))
    pool_o = ctx.enter_context(tc.tile_pool(name="o", bufs=6))
    pool_ps_xt = ctx.enter_context(tc.tile_pool(name="ps_xt", bufs=2, space="PSUM"))
    pool_ps = ctx.enter_context(tc.tile_pool(name="ps", bufs=2, space="PSUM"))
    if OUT_ROWMAJOR:
        pool_ps_oT = ctx.enter_context(tc.tile_pool(name="ps_oT", bufs=2, space="PSUM"))
        pool_out = ctx.enter_context(tc.tile_pool(name="oT", bufs=6))

    # Partition p of the x tile holds rows 16p..16p+15 (2KB contiguous DRAM).
    # After the PE transpose of chunk c, ps_xt[32q+d, 128c+n] = feature d of
    # row base + 16n + 4c + q, i.e. block b=4c+q covers rows {16n + b}.
    x_r = x.rearrange("(t p w) d -> t p (w d)", p=128, w=16)
    if OUT_ROWMAJOR:
        # oT[n, 64b+m] -> y[base + 16n + b, m]; per-partition 4KB contiguous
        y_r = y.rearrange("(t n b) m -> t n (b m)", n=128, b=16)
        # half-split: b = 4c + 2X + j -> [t][n][(c X j m)] sliced at X
        y_rh = y.rearrange("(t n c x j) m -> t x n c (j m)", n=128, c=4, x=2, j=2)
    else:
        # raw device layout [128, rows/2]; host unscrambles (see _run)
        y_r = y.rearrange("p (t f) -> t p f", f=1024)

    xa_d, xt_d, h1_d, h2_d, o_d = {}, {}, {}, {}, {}

    def st_load(t):
        xa = pool_in.tile([128, 512], MMDT, tag="xa")
        if MM_FAST:
            nc.gpsimd.dma_start(xa[:], x_r[t])
        else:
            nc.sync.dma_start(xa[:], x_r[t])
        xa_d[t] = xa

    def st_int(t):
        xa = xa_d.pop(t)
        xt = pool_xt.tile([128, 512], MMDT, tag="xt")
        if not DBG_SKIP_INT:
            ps_xt = pool_ps_xt.tile([128, 512], MMDT, tag="psxt")
            for c in range(4):
                nc.tensor.transpose(
                    ps_xt[:, 128 * c : 128 * (c + 1)],
                    xa[:, 128 * c : 128 * (c + 1)],
                    id_sb,
                )
            nc.vector.tensor_copy(xt[:], ps_xt[:])
        else:
            nc.vector.tensor_copy(xt[:], xa[:])
        xt_d[t] = xt

    def st_l1(t):
        xt = xt_d.pop(t)
        ps1 = pool_ps.tile([128, 1024], F32, tag="ps")
        nc.tensor.matmul(ps1[:, 0:512], w1_sb[0:64], xt[0:64, :], start=True, stop=True)
        nc.tensor.matmul(ps1[:, 512:1024], w1_sb[64:128], xt[64:128, :], start=True, stop=True)
        h1 = pool_h1.tile([128, 1024], MMDT, tag="h1")
        nc.scalar.activation(h1[:], ps1[:], AF.Relu, bias=b1_sb)
        h1_d[t] = h1

    def st_l2(t):
        h1 = h1_d.pop(t)
        ps2 = pool_ps.tile([128, 1024], F32, tag="ps")
        nc.tensor.matmul(ps2[:, 0:512], w2_sb, h1[:, 0:512], start=True, stop=True)
        nc.tensor.matmul(ps2[:, 512:1024], w2_sb, h1[:, 512:1024], start=True, stop=True)
        h2 = pool_h2.tile([128, 1024], MMDT, tag="h2")
        nc.scalar.activation(h2[:], ps2[:], AF.Relu, bias=b2_sb)
        h2_d[t] = h2

    def st_l3(t):
        h2 = h2_d.pop(t)
        ps3 = pool_ps.tile([128, 1024], F32, tag="ps")
        nc.tensor.matmul(ps3[:, 0:512], w3_sb, h2[:, 0:512], start=True, stop=True)
        nc.tensor.matmul(ps3[:, 512:1024], w3_sb, h2[:, 512:1024], start=True, stop=True)
        o = pool_o.tile([128, 1024], F32, tag="o")
        nc.vector.tensor_scalar_add(o[:], ps3[:], b3_sb)
        o_d[t] = o

    def st_out(t):
        o = o_d.pop(t)
        if OUT_ROWMAJOR and not DBG_SKIP_OUTT:
            # per X-half: transpose 4 chunks -> [128,512] psum, copy, DMA.
            # Half X covers b = 4c + 2X + j -> y rows 16n + b.
            for X in range(2):
                ps_oT = pool_ps_oT.tile([128, 512], F32, tag="psoT")
                for c in range(4):
                    nc.tensor.transpose(
                        ps_oT[:, 128 * c : 128 * (c + 1)],
                        o[:, 512 * X + 128 * c : 512 * X + 128 * (c + 1)],
                        id_f32,
                    )
                oT = pool_out.tile([128, 512], F32, tag="oT")
                if X == 0:
                    nc.scalar.copy(oT[:], ps_oT[:])
                else:
                    nc.vector.tensor_copy(oT[:], ps_oT[:])
                # oT[n, 128c + 64j + m] -> y[base + 16n + 4c + 2X + j, m]
                nc.sync.dma_start(y_rh[t, X], oT[:])
        else:
            nc.scalar.dma_start(y_r[t], o[:].bitcast(F32))

    stages = [st_load, st_int, st_l1, st_l2, st_l3, st_out]
    skew = [0, 1, 2, 3, 4, 5]
    for w in range(nst + skew[-1]):
        for s, fn in enumerate(stages):
            t = w - skew[s]
            if 0 <= t < nst:
                fn(t)


def _build_program(rows_per_core=ROWS_PER_CORE, repeats=1):
    nc = bacc.Bacc(
        "TRN2",
        target_bir_lowering=False,
        debug=False,
        num_devices=N_CORES,
    )
    x = nc.dram_tensor("x", [rows_per_core, D_IN], F32, kind="ExternalInput").ap()
    if OUT_ROWMAJOR:
        y = nc.dram_tensor("y", [rows_per_core, W], F32, kind="ExternalOutput").ap()
    else:
        y = nc.dram_tensor("y", [128, rows_per_core // 2], F32, kind="ExternalOutput").ap()
    cst = nc.dram_tensor("cst", [128, 515], F32, kind="ExternalInput").ap()
    aps = (x, y, cst, None, None, None, None, None, None)
    with tile.TileContext(nc) as tc:
        with ExitStack() as ctx:
            if repeats > 1:
                with tc.For_i(0, repeats, 1):
                    _emit(ctx, nc, tc, aps, rows_per_core)
            else:
                _emit(ctx, nc, tc, aps, rows_per_core)
    nc.compile()
    return nc


def _host_blocks(W1, b1, W2, b2, W3, b3):
    w1blk = np.zeros((128, 128), np.float32)
    w1blk[0:32, 0:64] = W1
    w1blk[32:64, 64:128] = W1
    w1blk[64:96, 0:64] = W1
    w1blk[96:128, 64:128] = W1
    w2blk = np.zeros((128, 128), np.float32)
    w2blk[0:64, 0:64] = W2
    w2blk[64:128, 64:128] = W2
    w3blk = np.zeros((128, 128), np.float32)
    w3blk[0:64, 0:64] = W3
    w3blk[64:128, 64:128] = W3
    ident = np.eye(128, dtype=np.float32)
    b1bc = np.concatenate([b1, b1])[:, None].astype(np.float32)
    b2bc = np.concatenate([b2, b2])[:, None].astype(np.float32)
    b3bc = np.concatenate([b3, b3])[:, None].astype(np.float32)
    cst = np.ascontiguousarray(
        np.concatenate([w1blk, w2blk, w3blk, ident, b1bc, b2bc, b3bc], axis=1)
    )
    return dict(cst=cst)


_NC_CACHE = {}


def _get_program(rows_per_core=ROWS_PER_CORE, repeats=1):
    key = (rows_per_core, MM_FAST, OUT_ROWMAJOR, repeats)
    if key not in _NC_CACHE:
        _NC_CACHE[key] = _build_program(rows_per_core, repeats=repeats)
    return _NC_CACHE[key]


def _run(inputs, rows_per_core=ROWS_PER_CORE, trace=False, trace_kwargs=None):
    nc = _get_program(rows_per_core)
    blocks = _host_blocks(
        inputs["W1"], inputs["b1"], inputs["W2"], inputs["b2"],
        inputs["W3"], inputs["b3"],
    )
    x = np.ascontiguousarray(np.asarray(inputs["x"], dtype=np.float32))
    n_rows = x.shape[0]
    assert n_rows == rows_per_core * N_CORES
    in_maps = []
    for c in range(N_CORES):
        m = dict(blocks)
        m["x"] = np.ascontiguousarray(x[c * rows_per_core : (c + 1) * rows_per_core])
        in_maps.append(m)
    kw = {}
    if trace:
        kw["trace"] = True
        if trace_kwargs:
            kw.update(trace_kwargs)
    last_err = None
    for _attempt in range(3):
        try:
            res = bass_utils.run_bass_kernel_spmd(
                nc, in_maps, core_ids=list(range(N_CORES)), **kw
            )
            break
        except Exception as e:  # transient device wedge -> retry
            last_err = e
            import time as _time

            _time.sleep(15)
    else:
        raise last_err
    outs = [r["y"] for r in res.results]
    if OUT_ROWMAJOR:
        full = np.concatenate(outs, axis=0)
    else:
        nst = rows_per_core // ST_ROWS
        decoded = []
        for o in outs:
            # o[64j+m, 1024t + 512X + 128c + n] = y[2048t + 16n + 4c + 2X + j, m]
            a = o.reshape(2, 64, nst, 2, 4, 128)  # j, m, t, X, c, n
            a = a.transpose(2, 5, 4, 3, 0, 1)  # t, n, c, X, j, m
            decoded.append(np.ascontiguousarray(a.reshape(rows_per_core, 64)))
        full = np.concatenate(decoded, axis=0)
    return np.ascontiguousarray(full.astype(np.float32)), res


def kernel(x, W1, b1, W2, b2, W3, b3):
    out, _ = _run(dict(x=x, W1=W1, b1=b1, W2=W2, b2=b2, W3=W3, b3=b3))
    return out
